# revision 1
# baseline (speedup 1.0000x reference)
"""DimeNet++ forward on 8 TRN2 NeuronCores (Bass/Tile).

Sharding: nodes -> 128-node chunks -> cores; edges/triplets co-located with the
owner of their destination node (METIS-style by dst). Per-core slot order
groups by (dst-chunk, wgroup of q-row source owner) with fixed pads so the
SPMD program is uniform across cores. Cross-core traffic: one bf16 AllToAll of
deduplicated q rows per interaction block plus one small bf16 AllGather of h.
Scatter-adds run as one-hot matmuls accumulated in PSUM per node chunk.

kernel(**inputs) takes the FULL unsharded inputs and returns [N, 1] float32.
"""
import os
import numpy as np

import concourse.bass as bass
import concourse.bacc as bacc
import concourse.mybir as mybir
import concourse.tile as tile
from concourse.bass import AP, IndirectOffsetOnAxis
from concourse.bass_utils import run_bass_kernel_spmd
from concourse.masks import make_identity

H = 128
NR = 16
NBLK = 4
CUTOFF = 5.0
F32 = mybir.dt.float32
BF16 = mybir.dt.bfloat16
I16 = mybir.dt.int16
AF = mybir.ActivationFunctionType


def _bessel(d):
    freq = np.arange(1, NR + 1, dtype=np.float32) * np.pi / CUTOFF
    p = 6
    a_, b_, c_ = -(p + 1) * (p + 2) / 2.0, p * (p + 2) * 1.0, -p * (p + 1) / 2.0
    x = d / CUTOFF
    env = (1.0 + a_ * x**p + b_ * x**(p + 1) + c_ * x**(p + 2)) * (d < CUTOFF)
    return (np.sqrt(2.0 / CUTOFF) * np.sin(freq * d[:, None]) / d[:, None] * env[:, None]).astype(np.float32)


def _spherical(dirs):
    x, y, z = dirs[:, 0], dirs[:, 1], dirs[:, 2]
    return np.stack([np.ones_like(x), y, z, x, x * y, y * z, 3 * z**2 - 1], -1).astype(np.float32)


def _sigmoid(x):
    return 1.0 / (1.0 + np.exp(-x))


class Cfg:
    def __init__(self, ncores=8, nchk=10, cpw=2304, wpad=3072):
        self.NCORES = ncores
        self.NCHK = nchk
        self.CPW = cpw
        self.CHUNK_CAP = 2 * cpw
        self.ES = nchk * self.CHUNK_CAP
        self.TILES = self.ES // 128
        self.TPC = self.CHUNK_CAP // 128       # tiles per chunk
        self.CPW_T = cpw // 128                # tiles per (chunk, w)
        self.WPAD = wpad
        self.SHARD = 2 * wpad
        self.XBUF = ncores * self.SHARD
        self.NLOC = nchk * 128
        self.NODE_PAD = ncores * self.NLOC
        self.G512 = self.ES // 512
        assert cpw % 128 == 0 and wpad % 128 == 0 and self.ES % 512 == 0
        assert self.ES // 2 <= 32767
        assert (ncores // 2) * self.SHARD <= 32768


def _wrap16_block(flat):
    """[n] -> [128, n//16] int16 idx layout for dma_gather (16-wrap, 8x replicated)."""
    n = flat.shape[0]
    arr = np.zeros((128, n // 16), dtype=np.int16)
    arr[:16, :] = flat.reshape(n // 16, 16).T
    arr[:] = np.tile(arr[:16, :], (8, 1))
    return arr


def build_plan(inp, cfg):
    C = cfg
    Z = inp['atomic_numbers']
    src = inp['edge_index'][0].astype(np.int64)
    dst = inp['edge_index'][1].astype(np.int64)
    idx_ji = inp['triplets'][:, 0].astype(np.int64)
    idx_kj = inp['triplets'][:, 1].astype(np.int64)
    d = inp['edge_distances']
    N = Z.shape[0]
    E = d.shape[0]

    rbf = _bessel(d)
    sbf = _spherical(inp['edge_directions'])
    a_all = np.stack([
        _sigmoid(np.mean(sbf @ inp['ib_sph_w'][i] + inp['ib_sph_b'][i], -1))
        for i in range(NBLK)])
    h0 = inp['atom_emb'][Z - 1].astype(np.float32)

    chunk_of_node = np.arange(C.NODE_PAD) // 128
    core_of_chunk = chunk_of_node // C.NCHK
    owner_e = core_of_chunk[dst]
    wgrp_of_core = (np.arange(C.NCORES) >= C.NCORES // 2).astype(np.int64)

    sigma = np.full((C.NCORES, C.ES), -1, dtype=np.int64)
    slot_of_edge = np.full(E, -1, dtype=np.int64)
    for c in range(C.NCORES):
        mine = np.where(owner_e == c)[0]
        lchunk = chunk_of_node[dst[mine]] - c * C.NCHK
        wg = wgrp_of_core[owner_e[idx_kj[mine]]]
        order = np.lexsort((mine, wg, lchunk))
        mine, lchunk, wg = mine[order], lchunk[order], wg[order]
        for k in range(C.NCHK):
            for w in range(2):
                grp = mine[(lchunk == k) & (wg == w)]
                assert grp.shape[0] <= C.CPW, (c, k, w, grp.shape[0], C.CPW)
                base = k * C.CHUNK_CAP + w * C.CPW
                sigma[c, base:base + grp.shape[0]] = grp
                slot_of_edge[grp] = base + np.arange(grp.shape[0])

    HALF = C.ES // 2
    stage_idx = np.zeros((C.NCORES, C.NCORES, 2, C.WPAD), dtype=np.int64)
    rowpos_lut = np.full(E, -1, dtype=np.int64)  # reused per (o,c) pair
    recvpos = np.zeros((C.NCORES, C.ES), dtype=np.int64)
    req_src = [idx_kj[np.clip(sigma[c], 0, None)] for c in range(C.NCORES)]
    req_valid = [sigma[c] >= 0 for c in range(C.NCORES)]
    for o in range(C.NCORES):
        for c in range(C.NCORES):
            sel = req_valid[c] & (owner_e[req_src[c]] == o)
            pos = np.unique(slot_of_edge[req_src[c][sel]])
            lo = pos[pos < HALF]
            hi = pos[pos >= HALF]
            assert lo.shape[0] <= C.WPAD and hi.shape[0] <= C.WPAD, (o, c, lo.shape[0], hi.shape[0])
            stage_idx[o, c, 0, :lo.shape[0]] = lo
            stage_idx[o, c, 1, :hi.shape[0]] = hi - HALF
            # row position within the (o->c) shard for each staged slot
            rowpos_lut[:] = -1
            # map via slot ids: build slot->row map array of size ES
            srow = np.full(C.ES, -1, dtype=np.int64)
            srow[lo] = np.arange(lo.shape[0])
            srow[hi] = C.WPAD + np.arange(hi.shape[0])
            rp = srow[slot_of_edge[req_src[c][sel]]]
            assert (rp >= 0).all()
            recvpos[c, np.where(sel)[0]] = o * C.SHARD + rp

    WHALF = (C.NCORES // 2) * C.SHARD
    plans = []
    for c in range(C.NCORES):
        sl = sigma[c]
        valid = sl >= 0
        slc = np.clip(sl, 0, None)
        rbfT = np.where(valid[None, :], rbf[slc].T, 0.0).astype(np.float32)
        ag = np.where(valid[None, :], a_all[:, idx_ji[slc]], 0.0).astype(np.float32)
        dloc = np.where(valid, dst[slc] - (c * C.NCHK * 128 + (np.arange(C.ES) // C.CHUNK_CAP) * 128), 0)
        S = np.zeros((C.ES, 128), dtype=np.float32)
        S[np.arange(C.ES)[valid], dloc[valid]] = 1.0
        hsrc = np.where(valid, src[slc], 0).astype(np.int64)
        cons = recvpos[c].copy()
        for k in range(C.NCHK):
            w1 = slice(k * C.CHUNK_CAP + C.CPW, k * C.CHUNK_CAP + 2 * C.CPW)
            cons[w1] = cons[w1] - WHALF
        cons = np.clip(cons, 0, None)

        # device layouts
        ag_dev = np.zeros((128, NBLK * C.TILES), np.float32)
        for i in range(NBLK):
            ag_dev[:, i * C.TILES:(i + 1) * C.TILES] = ag[i].reshape(C.TILES, 128).T
        S_dev = np.zeros((128, C.TILES * 128), np.float32)
        for t in range(C.TILES):
            S_dev[:, t * 128:(t + 1) * 128] = S[t * 128:(t + 1) * 128]
        stage_dev = np.concatenate(
            [_wrap16_block(stage_idx[c, c2, w]) for c2 in range(C.NCORES) for w in range(2)], axis=1
        ).astype(np.int16)
        cons_dev = np.concatenate(
            [_wrap16_block(cons[k * C.CHUNK_CAP + w * C.CPW: k * C.CHUNK_CAP + (w + 1) * C.CPW])
             for k in range(C.NCHK) for w in range(2)], axis=1).astype(np.int16)
        hg_dev = np.concatenate(
            [_wrap16_block(hsrc[k * C.CHUNK_CAP:(k + 1) * C.CHUNK_CAP])
             for k in range(C.NCHK)], axis=1).astype(np.int16)
        h0_pad = np.zeros((C.NODE_PAD, H), np.float32)
        h0_pad[:N] = h0
        h0T_loc = h0_pad[c * C.NLOC:(c + 1) * C.NLOC].T.copy().astype(np.float32)
        plans.append(dict(
            sigma=sl, rbfT=rbfT.astype(np.float32), ag=ag_dev, S=S_dev,
            stage=stage_dev, cons=cons_dev, hg=hg_dev, h0T=h0T_loc,
        ))
    h0tab = np.zeros((C.NODE_PAD, H), np.float32)
    h0tab[:N] = h0
    meta = dict(h0tab=h0tab, N=N)
    return plans, meta


def pack_weights(inp):
    """Stack all matmul weights into wmat [NW,128,128] bf16-able f32 and biases."""
    NW, NV = 63, 54
    wmat = np.zeros((NW, 128, 128), np.float32)
    wvec = np.zeros((128, NV), np.float32)
    wrow = np.zeros((128, 5 * 128), np.float32)
    wmat[0, :NR, :] = inp['embed_w'][:, 2 * H:]
    wvec[:, 0] = inp['embed_b'][2 * H:]
    for i in range(NBLK):
        b = 1 + 8 * i
        wmat[b + 0, :NR, :] = inp['ib_rad_w1'][i]
        wmat[b + 1] = inp['ib_rad_w2'][i]
        wmat[b + 2] = inp['ib_upd_w1'][i][:128]
        wmat[b + 3] = inp['ib_upd_w1'][i][128:]
        wmat[b + 4] = inp['ib_upd_w2'][i]
        wmat[b + 5] = inp['ib_out_w'][i, 0]
        wmat[b + 6] = inp['ib_out_w'][i, 1]
        wmat[b + 7] = inp['ib_out_w'][i, 2]
        v = 1 + 7 * i
        wvec[:, v + 0] = inp['ib_rad_b1'][i]
        wvec[:, v + 1] = inp['ib_rad_b2'][i]
        wvec[:, v + 2] = inp['ib_upd_b1'][i]
        wvec[:, v + 3] = inp['ib_upd_b2'][i]
        wvec[:, v + 4] = inp['ib_out_b'][i, 0]
        wvec[:, v + 5] = inp['ib_out_b'][i, 1]
        wvec[:, v + 6] = inp['ib_out_b'][i, 2]
    for i in range(NBLK + 1):
        b = 33 + 5 * i
        wmat[b + 0, :NR, :] = inp['ob_rad_w1'][i]
        wmat[b + 1] = inp['ob_rad_w2'][i]
        wmat[b + 2] = inp['ob_dense_w'][i, 0]
        wmat[b + 3] = inp['ob_dense_w'][i, 1]
        wmat[b + 4] = inp['ob_dense_w'][i, 2]
        wmat[58 + i, :, 0] = inp['ob_out_w'][i][:, 0]
        v = 29 + 5 * i
        wvec[:, v + 0] = inp['ob_rad_b1'][i]
        wvec[:, v + 1] = inp['ob_dense_b'][i, 0]
        wvec[:, v + 2] = inp['ob_dense_b'][i, 1]
        wvec[:, v + 3] = inp['ob_dense_b'][i, 2]
        # v+4 spare
        wrow[:, i * 128:(i + 1) * 128] = inp['ob_rad_b2'][i][None, :]
    out_bias = float(inp['ob_out_b'].sum())
    # wmat -> [128, NW*128] device layout
    wm_dev = np.zeros((128, NW * 128), np.float32)
    for wI in range(NW):
        wm_dev[:, wI * 128:(wI + 1) * 128] = wmat[wI]
    return wm_dev, wvec, wrow, out_bias


# weight slot index helpers
def W_EMB():
    return 0
def W_IBR1(i):
    return 1 + 8 * i
def W_IBR2(i):
    return 2 + 8 * i
def W_UPDA(i):
    return 3 + 8 * i
def W_UPDB(i):
    return 4 + 8 * i
def W_UPDW2(i):
    return 5 + 8 * i
def W_IBOUT(i, j):
    return 6 + 8 * i + j
def W_OBR1(i):
    return 33 + 5 * i
def W_OBR2(i):
    return 34 + 5 * i
def W_OBD(i, j):
    return 35 + 5 * i + j
def W_OBOUT(i):
    return 58 + i
def V_EMB():
    return 0
def V_IBR1(i):
    return 1 + 7 * i
def V_IBR2(i):
    return 2 + 7 * i
def V_UPD1(i):
    return 3 + 7 * i
def V_UPD2(i):
    return 4 + 7 * i
def V_IBOUT(i, j):
    return 5 + 7 * i + j
def V_OBR1(i):
    return 29 + 5 * i
def V_OBD(i, j):
    return 30 + 5 * i + j


def build_bass(C):
    NW, NV = 63, 54
    nc = bacc.Bacc("TRN2", target_bir_lowering=False, debug=False, num_devices=C.NCORES)

    # ---- external I/O ----
    ext = {}
    ext['rbfT'] = nc.declare_dram_parameter("rbfT", [NR, C.ES], BF16, isOutput=False)
    ext['ag'] = nc.declare_dram_parameter("ag", [128, NBLK * C.TILES], F32, isOutput=False)
    ext['S'] = nc.declare_dram_parameter("S", [128, C.TILES * 128], BF16, isOutput=False)
    ext['stage_idx'] = nc.declare_dram_parameter("stage_idx", [128, C.NCORES * 2 * (C.WPAD // 16)], I16, isOutput=False)
    ext['cons_idx'] = nc.declare_dram_parameter("cons_idx", [128, C.NCHK * 2 * (C.CPW // 16)], I16, isOutput=False)
    ext['hg_idx'] = nc.declare_dram_parameter("hg_idx", [128, C.NCHK * (C.CHUNK_CAP // 16)], I16, isOutput=False)
    ext['h0T'] = nc.declare_dram_parameter("h0T", [128, C.NLOC], F32, isOutput=False)
    ext['h0tab'] = nc.declare_dram_parameter("h0tab", [C.NODE_PAD, H], BF16, isOutput=False)
    ext['wmat'] = nc.declare_dram_parameter("wmat", [128, NW * 128], BF16, isOutput=False)
    ext['wvec'] = nc.declare_dram_parameter("wvec", [128, NV], F32, isOutput=False)
    ext['wrow'] = nc.declare_dram_parameter("wrow", [128, 5 * 128], F32, isOutput=False)
    out_ext = nc.declare_dram_parameter("out", [1, C.NLOC], F32, isOutput=True)

    # ---- internal DRAM ----
    mT = nc.dram_tensor("mT", [128, C.ES], BF16)
    qtab = nc.dram_tensor("qtab", [C.ES, H], BF16)
    sendb = nc.dram_tensor("sendb", [C.XBUF, H], BF16)
    recvb = nc.dram_tensor("recvb", [C.XBUF, H], BF16)
    hloc = nc.dram_tensor("hloc", [C.NLOC, H], BF16)
    htab = nc.dram_tensor("htab", [C.NODE_PAD, H], BF16, addr_space="Shared")

    rg = [list(range(C.NCORES))]
    HALFE = C.ES // 2
    WHALF = (C.NCORES // 2) * C.SHARD

    with tile.TileContext(nc, num_cores=C.NCORES) as tc:
        with (
            tc.tile_pool(name="const", bufs=1) as cp,
            tc.tile_pool(name="psum", bufs=2, space="PSUM") as pp,
            tc.tile_pool(name="work", bufs=2) as wp,
            tc.tile_pool(name="gat", bufs=2) as gp,
            tc.tile_pool(name="acc", bufs=1) as ap_,
        ):
            # ---- persistent constants ----
            ag_sb = cp.tile([128, NBLK * C.TILES], F32, tag="ag")
            nc.sync.dma_start(ag_sb[:], ext['ag'][:, :])
            st_sb = cp.tile([128, C.NCORES * 2 * (C.WPAD // 16)], I16, tag="sti")
            nc.sync.dma_start(st_sb[:], ext['stage_idx'][:, :])
            co_sb = cp.tile([128, C.NCHK * 2 * (C.CPW // 16)], I16, tag="coi")
            nc.sync.dma_start(co_sb[:], ext['cons_idx'][:, :])
            hg_sb = cp.tile([128, C.NCHK * (C.CHUNK_CAP // 16)], I16, tag="hgi")
            nc.sync.dma_start(hg_sb[:], ext['hg_idx'][:, :])
            wm = cp.tile([128, NW * 128], BF16, tag="wm")
            nc.sync.dma_start(wm[:], ext['wmat'][:, :])
            wv = cp.tile([128, NV], F32, tag="wv")
            nc.sync.dma_start(wv[:], ext['wvec'][:, :])
            wr = cp.tile([128, 5 * 128], F32, tag="wr")
            nc.sync.dma_start(wr[:], ext['wrow'][:, :])
            ident = cp.tile([128, 128], BF16, tag="id")
            make_identity(nc, ident[:])
            hT0 = cp.tile([128, C.NLOC], F32, tag="hT0")
            hT1 = cp.tile([128, C.NLOC], F32, tag="hT1")
            hT = [hT0, hT1]
            nc.sync.dma_start(hT[0][:], ext['h0T'][:, :])
            out_sb = ap_.tile([1, C.NLOC], F32, tag="out")
            nc.vector.memset(out_sb[:], 0.0)
            agg_sb = ap_.tile([128, C.NLOC], F32, tag="agg")
            x_sb = ap_.tile([128, C.NLOC], F32, tag="x")

            def WMAT(wI, k=128):
                return wm[:k, wI * 128:(wI + 1) * 128]

            def rbf_chunk(e0, n=512):
                rc = gp.tile([NR, 512], BF16, tag="rbft")
                nc.sync.dma_start(rc[:, :n], ext['rbfT'][:, e0:e0 + n])
                return rc

            def BIAS(vI):
                return wv[:, vI:vI + 1]

            def n3groups():
                # node-dim 512-groups covering NLOC
                gs = []
                o = 0
                while o < C.NLOC:
                    gs.append((o, min(512, C.NLOC - o)))
                    o += 512
                return gs

            # ---- m init: m = rbf @ embed_w_m + b ----
            for g in range(C.G512):
                ps = pp.tile([128, 512], F32, tag="psA")
                rc = rbf_chunk(g * 512)
                nc.tensor.matmul(ps[:], WMAT(W_EMB(), NR), rc[:], start=True, stop=True)
                mt = wp.tile([128, 512], BF16, tag="mtile")
                nc.scalar.activation(mt[:], ps[:], AF.Identity, bias=BIAS(V_EMB()))
                nc.sync.dma_start(mT[:, g * 512:(g + 1) * 512], mt[:])

            def ob_block(i, htab_ap, h_cur):
                """output block i using h table AP (node-major bf16)."""
                for k in range(C.NCHK):
                    hg_t = gp.tile([128, C.TPC * H], BF16, tag="hg")
                    nc.gpsimd.dma_gather(
                        out_ap=hg_t[:].rearrange("p (j h) -> p j h", h=H),
                        in_ap=htab_ap,
                        idxs_ap=hg_sb[:, k * (C.CHUNK_CAP // 16):(k + 1) * (C.CHUNK_CAP // 16)],
                        num_idxs=C.CHUNK_CAP, num_idxs_reg=C.CHUNK_CAP, elem_size=H,
                        single_packet=False,
                    )
                    psx = pp.tile([128, 128], F32, tag="psAgg")
                    for gg in range(C.TPC // 4):
                        e0 = k * C.CHUNK_CAP + gg * 512
                        ps1 = pp.tile([128, 512], F32, tag="psA")
                        rc = rbf_chunk(e0)
                        nc.tensor.matmul(ps1[:], WMAT(W_OBR1(i), NR), rc[:], start=True, stop=True)
                        o1 = wp.tile([128, 512], BF16, tag="o1")
                        nc.scalar.activation(o1[:], ps1[:], AF.Silu, bias=BIAS(V_OBR1(i)))
                        for t4 in range(4):
                            t = gg * 4 + t4
                            pse = pp.tile([128, 128], F32, tag="psT")
                            nc.tensor.matmul(pse[:], o1[:, t4 * 128:(t4 + 1) * 128], WMAT(W_OBR2(i)), start=True, stop=True)
                            he = wp.tile([128, 128], F32, tag="he")
                            nc.vector.tensor_add(he[:], pse[:], wr[:, i * 128:(i + 1) * 128])
                            heb = wp.tile([128, 128], BF16, tag="heb")
                            nc.vector.tensor_mul(heb[:], he[:], hg_t[:, t * H:(t + 1) * H])
                            nc.tensor.matmul(
                                psx[:], heb[:], ext_S_tile(k * C.TPC + t),
                                start=(t == 0), stop=(t == C.TPC - 1), skip_group_check=True)
                    nc.scalar.activation(x_sb[:, k * 128:(k + 1) * 128], psx[:], AF.Copy)
                # dense layers on x (feat-major), then out projection
                for (o, n) in n3groups():
                    cur = wp.tile([128, 512], BF16, tag="xd")
                    nc.vector.tensor_copy(cur[:, :n], x_sb[:, o:o + n])
                    for j in range(3):
                        psd = pp.tile([128, 512], F32, tag="psA")
                        nc.tensor.matmul(psd[:, :n], WMAT(W_OBD(i, j)), cur[:, :n], start=True, stop=True)
                        nc.scalar.activation(cur[:, :n], psd[:, :n], AF.Silu, bias=BIAS(V_OBD(i, j)))
                    pso = pp.tile([1, 512], F32, tag="psO")
                    nc.tensor.matmul(pso[:, :n], WMAT(W_OBOUT(i))[:, 0:1], cur[:, :n], start=True, stop=True)
                    nc.vector.tensor_add(out_sb[:, o:o + n], out_sb[:, o:o + n], pso[:, :n])

            # S tiles are streamed from DRAM per chunk
            S_cache = {}

            def ext_S_tile(t):
                # S tile for global tile index t, loaded per chunk group
                k = t // C.TPC
                if k not in S_cache:
                    st = gp.tile([128, C.TPC * 128], BF16, tag="Sst")
                    nc.sync.dma_start(st[:], ext['S'][:, k * C.TPC * 128:(k + 1) * C.TPC * 128])
                    S_cache.clear()
                    S_cache[k] = st
                return S_cache[k][:, (t % C.TPC) * 128:(t % C.TPC + 1) * 128]

            ob_block(0, ext['h0tab'][:, :], hT[0])

            for i in range(NBLK):
                # ---- phase A: rad chain, q = m*rad, transpose to qtab ----
                for g in range(C.G512):
                    ps1 = pp.tile([128, 512], F32, tag="psA")
                    rc = rbf_chunk(g * 512)
                    nc.tensor.matmul(ps1[:], WMAT(W_IBR1(i), NR), rc[:], start=True, stop=True)
                    r1 = wp.tile([128, 512], BF16, tag="r1")
                    nc.scalar.activation(r1[:], ps1[:], AF.Silu, bias=BIAS(V_IBR1(i)))
                    ps2 = pp.tile([128, 512], F32, tag="psA")
                    nc.tensor.matmul(ps2[:], WMAT(W_IBR2(i)), r1[:], start=True, stop=True)
                    r2 = wp.tile([128, 512], BF16, tag="r2")
                    nc.scalar.activation(r2[:], ps2[:], AF.Identity, bias=BIAS(V_IBR2(i)))
                    mt = wp.tile([128, 512], BF16, tag="mtile")
                    nc.sync.dma_start(mt[:], mT[:, g * 512:(g + 1) * 512])
                    qf = wp.tile([128, 512], BF16, tag="qf")
                    nc.vector.tensor_mul(qf[:], mt[:], r2[:])
                    qe = wp.tile([128, 4 * 128], BF16, tag="qe")
                    for t4 in range(4):
                        pst = pp.tile([128, 128], F32, tag="psT")
                        nc.tensor.matmul(pst[:], qf[:, t4 * 128:(t4 + 1) * 128], ident[:], start=True, stop=True)
                        nc.scalar.activation(qe[:, t4 * 128:(t4 + 1) * 128], pst[:], AF.Copy)
                    nc.sync.dma_start(
                        qtab.ap().rearrange("(j p) h -> p j h", p=128)[:, g * 4:(g + 1) * 4, :],
                        qe[:].rearrange("p (j h) -> p j h", h=H))
                # ---- phase B: staging gathers + AllToAll ----
                for c2 in range(C.NCORES):
                    for w in range(2):
                        stg = gp.tile([128, (C.WPAD // 128) * H], BF16, tag="stg")
                        icol = (c2 * 2 + w) * (C.WPAD // 16)
                        nc.gpsimd.dma_gather(
                            out_ap=stg[:].rearrange("p (j h) -> p j h", h=H),
                            in_ap=qtab[w * HALFE:(w + 1) * HALFE, :],
                            idxs_ap=st_sb[:, icol:icol + C.WPAD // 16],
                            num_idxs=C.WPAD, num_idxs_reg=C.WPAD, elem_size=H,
                            single_packet=False,
                        )
                        nc.sync.dma_start(
                            sendb.ap().rearrange("(j p) h -> p j h", p=128)[
                                :, (c2 * C.SHARD + w * C.WPAD) // 128:(c2 * C.SHARD + (w + 1) * C.WPAD) // 128, :],
                            stg[:].rearrange("p (j h) -> p j h", h=H))
                nc.gpsimd.collective_compute(
                    "AllToAll", mybir.AluOpType.bypass, replica_groups=rg,
                    ins=[sendb.ap().opt()], outs=[recvb.ap().opt()])
                # ---- phase C: per chunk triplet stage ----
                for k in range(C.NCHK):
                    consw = []
                    for w in range(2):
                        ct = gp.tile([128, C.CPW_T * H], BF16, tag="cons")
                        icol = (k * 2 + w) * (C.CPW // 16)
                        nc.gpsimd.dma_gather(
                            out_ap=ct[:].rearrange("p (j h) -> p j h", h=H),
                            in_ap=recvb[w * WHALF:, :] if w == 1 else recvb[0:WHALF, :],
                            idxs_ap=co_sb[:, icol:icol + C.CPW // 16],
                            num_idxs=C.CPW, num_idxs_reg=C.CPW, elem_size=H,
                            single_packet=False,
                        )
                        consw.append(ct)
                    mkfT = ap_.tile([128, C.CHUNK_CAP], BF16, tag="mkfT")
                    psg = pp.tile([128, 128], F32, tag="psAgg")
                    for t in range(C.TPC):
                        srct = consw[0] if t < C.CPW_T else consw[1]
                        toff = (t if t < C.CPW_T else t - C.CPW_T) * H
                        mkf = wp.tile([128, 128], BF16, tag="mkf")
                        nc.vector.tensor_scalar_mul(
                            mkf[:], srct[:, toff:toff + H],
                            ag_sb[:, i * C.TILES + k * C.TPC + t: i * C.TILES + k * C.TPC + t + 1])
                        nc.tensor.matmul(psg[:], mkf[:], ext_S_tile(k * C.TPC + t),
                                         start=(t == 0), stop=(t == C.TPC - 1), skip_group_check=True)
                        pstr = pp.tile([128, 128], F32, tag="psT")
                        nc.tensor.matmul(pstr[:], mkf[:], ident[:], start=True, stop=True)
                        nc.scalar.activation(mkfT[:, t * 128:(t + 1) * 128], pstr[:], AF.Copy)
                    nc.scalar.activation(agg_sb[:, k * 128:(k + 1) * 128], psg[:], AF.Copy)
                    # m_new for this chunk + m update
                    for gg in range(C.TPC // 4):
                        e0 = k * C.CHUNK_CAP + gg * 512
                        mn = wp.tile([128, 512], F32, tag="mn")
                        for j in range(3):
                            psj = pp.tile([128, 512], F32, tag="psA")
                            nc.tensor.matmul(psj[:], WMAT(W_IBOUT(i, j)), mkfT[:, gg * 512:(gg + 1) * 512], start=True, stop=True)
                            if j == 0:
                                nc.scalar.activation(mn[:], psj[:], AF.Silu, bias=BIAS(V_IBOUT(i, j)))
                            else:
                                sj = wp.tile([128, 512], F32, tag="sj")
                                nc.scalar.activation(sj[:], psj[:], AF.Silu, bias=BIAS(V_IBOUT(i, j)))
                                nc.vector.tensor_add(mn[:], mn[:], sj[:])
                        mold = wp.tile([128, 512], BF16, tag="mtile")
                        nc.sync.dma_start(mold[:], mT[:, e0:e0 + 512])
                        mnew = wp.tile([128, 512], BF16, tag="mnew")
                        nc.vector.tensor_add(mnew[:], mn[:], mold[:])
                        nc.sync.dma_start(mT[:, e0:e0 + 512], mnew[:])
                # ---- phase D: h update + allgather ----
                cur, nxt = hT[i % 2], hT[(i + 1) % 2]
                for (o, n) in n3groups():
                    hb = wp.tile([128, 512], BF16, tag="hb")
                    nc.vector.tensor_copy(hb[:, :n], cur[:, o:o + n])
                    ab = wp.tile([128, 512], BF16, tag="ab")
                    nc.vector.tensor_copy(ab[:, :n], agg_sb[:, o:o + n])
                    psu = pp.tile([128, 512], F32, tag="psA")
                    nc.tensor.matmul(psu[:, :n], WMAT(W_UPDA(i)), hb[:, :n], start=True, stop=False)
                    nc.tensor.matmul(psu[:, :n], WMAT(W_UPDB(i)), ab[:, :n], start=False, stop=True)
                    u1 = wp.tile([128, 512], BF16, tag="u1")
                    nc.scalar.activation(u1[:, :n], psu[:, :n], AF.Silu, bias=BIAS(V_UPD1(i)))
                    psu2 = pp.tile([128, 512], F32, tag="psA")
                    nc.tensor.matmul(psu2[:, :n], WMAT(W_UPDW2(i)), u1[:, :n], start=True, stop=True)
                    ud = wp.tile([128, 512], F32, tag="ud")
                    nc.scalar.activation(ud[:, :n], psu2[:, :n], AF.Identity, bias=BIAS(V_UPD2(i)))
                    nc.vector.tensor_add(nxt[:, o:o + n], cur[:, o:o + n], ud[:, :n])
                # transpose h shard to node-major bf16 + allgather
                for k in range(C.NCHK):
                    hbt = wp.tile([128, 128], BF16, tag="hbt")
                    nc.vector.tensor_copy(hbt[:], nxt[:, k * 128:(k + 1) * 128])
                    psh = pp.tile([128, 128], F32, tag="psT")
                    nc.tensor.matmul(psh[:], hbt[:], ident[:], start=True, stop=True)
                    hnm = wp.tile([128, 128], BF16, tag="hnm")
                    nc.scalar.activation(hnm[:], psh[:], AF.Copy)
                    nc.sync.dma_start(hloc[k * 128:(k + 1) * 128, :], hnm[:])
                nc.gpsimd.collective_compute(
                    "AllGather", mybir.AluOpType.bypass, replica_groups=rg,
                    ins=[hloc.ap().opt()], outs=[htab.ap().opt()])
                ob_block(i + 1, htab[:, :], hT[(i + 1) % 2])

            nc.sync.dma_start(out_ext[:, :], out_sb[:])
    nc.compile()
    return nc


_CACHE = {}


def _get_nc(C):
    key = (C.NCORES, C.NCHK, C.CPW, C.WPAD)
    if key not in _CACHE:
        _CACHE[key] = build_bass(C)
    return _CACHE[key]


def make_in_maps(inp, C):
    import ml_dtypes
    bf = ml_dtypes.bfloat16
    plans, meta = build_plan(inp, C)
    wm_dev, wvec, wrow, out_bias = pack_weights(inp)
    in_maps = []
    for c in range(C.NCORES):
        p = plans[c]
        in_maps.append({
            "rbfT": p['rbfT'].astype(bf),
            "ag": p['ag'].astype(np.float32),
            "S": p['S'].astype(bf),
            "stage_idx": p['stage'],
            "cons_idx": p['cons'],
            "hg_idx": p['hg'],
            "h0T": p['h0T'].astype(np.float32),
            "h0tab": meta['h0tab'].astype(bf),
            "wmat": wm_dev.astype(bf),
            "wvec": wvec.astype(np.float32),
            "wrow": wrow.astype(np.float32),
        })
    return in_maps, meta, out_bias


def run(inp, C, sim=False):
    inp = {k: np.asarray(v) for k, v in inp.items()}
    N = inp['atomic_numbers'].shape[0]
    in_maps, meta, out_bias = make_in_maps(inp, C)
    nc = _get_nc(C)
    exec_ns = None
    if sim:
        import concourse.bass_interp as bass_interp
        s = bass_interp.MultiCoreSim(nc, C.NCORES)
        for c in range(C.NCORES):
            for k, v in in_maps[c].items():
                t = s.cores[c].tensor(k)
                t[:] = v.astype(t.dtype) if t.dtype != v.dtype else v
        s.simulate(check_with_hw=False)
        results = [{"out": s.cores[c].mem_tensor("out")} for c in range(C.NCORES)]
    else:
        res = run_bass_kernel_spmd(nc, in_maps, core_ids=list(range(C.NCORES)),
                                   trace=bool(int(os.environ.get("TRACE", "0"))))
        results = res.results
        exec_ns = res.exec_time_ns
    out = np.zeros((C.NODE_PAD,), np.float32)
    for c in range(C.NCORES):
        out[c * C.NLOC:(c + 1) * C.NLOC] = np.asarray(results[c]["out"], np.float32)[0]
    run._last_exec_ns = exec_ns
    return (out[:N] + out_bias)[:, None].astype(np.float32)


def kernel(**inputs):
    C = Cfg(ncores=8, nchk=10, cpw=2304, wpad=3072)
    out = run(inputs, C, sim=False)
    kernel._last_exec_ns = run._last_exec_ns
    return out

